# revision 4
# baseline (speedup 1.0000x reference)
"""Distributed Trainium2 kernel for nn_AncProbsLayer.

Math (reference):
    tau[b,h]  = softplus(tau_kernel[h, rate_indices[b,h]])
    R,p,Q     from tiny (H,K,20,20) kernels; Sm = D^1/2 Q D^-1/2; lam,U = eigh(Sm)
    P[b,h,k]  = D^-1/2 U diag(exp(tau*lam)) U^T D^1/2
    out       = einsum('blhz,bhkzs->blhks', inputs, P)

Key structural facts exploited:
  * `inputs` is ONE-HOT, so the contraction has exactly one nonzero term
    per h-block: out rows are just rows of P_comb[b] = BDV diag(E_b) BDW.
    A single bf16 matmul reproduces them to ~0.2% (gate is 2e-2
    absmax-relative); no hi/lo splitting is needed, and the one-hot moving
    operand is EXACT in fp8 (values 0/1), halving input DMA bytes.
  * All the tiny eigen/tau/P_comb math runs on HOST (float64) — only the
    streaming gather-matmul runs on device. Per pair of batches (even b at
    image partitions 0-39, odd at 40-79) two matmuls with an [80,80]
    block-diagonal stationary (pc_even ⊕ pc_odd, split in two 40-feature
    halves) produce the full [80-feature, 512-position] output tile.
  * Output is uint8-quantized on-chip (u8 = 250*x + 4.5, decoded on host);
    P entries lie in [~-0.01, 0.99] so the quant error is ~0.2% absmax.

Distribution: data-parallel over batch B across 8 cores (128 b each), no
collectives. Per core per pass: in-DMA 2.62 MB (fp8 one-hot image), out-DMA
5.24 MB (u8), PE 2x64 matmuls x 512 moving rows = 65536 cycles ~= 27.3 us
@2.4 GHz -- the expected bottleneck. PSUM is evacuated in per-pair
[80,1024] tiles split across DVE/ACT/Pool.
"""

import numpy as np
import ml_dtypes

import concourse.bass as bass
import concourse.bacc as bacc
import concourse.mybir as mybir
from concourse.tile import TileContext
from concourse.bass_utils import run_bass_kernel_spmd

# Problem constants (hardcoded per the harness contract)
B, L, H, K, S = 1024, 512, 2, 2, 20
NCORES = 8
BPC = B // NCORES          # 128 batches per core
NP = BPC // 2              # 64 even/odd pairs per core
COLS = NP * L              # 32768 image columns per core
F = H * K * S              # 80 output features
CD = 2 * H * S             # 80 contraction rows (even 0-39, odd 40-79)
GP = 8                     # pairs per DMA group
NG = NP // GP              # 8 groups

F32 = mybir.dt.float32
BF16 = mybir.dt.bfloat16
U8 = mybir.dt.uint8
NPBF16 = np.dtype(ml_dtypes.bfloat16)
NPF8E4 = np.dtype(ml_dtypes.float8_e4m3)

IMG_FP8 = True             # one-hot moving image dtype: fp8e4 (exact) vs bf16
OUT_U8 = True              # on-chip u8 quantization of the output
QSCALE = 250.0
QBIAS = 4.5
DEC_BIAS = 4.25            # host decode offset (covers trunc or RNE convert)

# evac column split within each 512-col half: DVE | ACT (GPSIMD/Pool
# cannot read PSUM)
EV_DVE = 328
EV_ACT = 184               # 328 + 184 = 512
OG = 2                     # groups per out-DMA

_NC_CACHE = {}


def build_nc(reps=1):
    # reps>1 repeats the main stream inside one NEFF (benchmarking only:
    # (wall[R] - wall[1])/(R-1) cancels dispatch overhead exactly)
    nc = bacc.Bacc(
        "TRN2", target_bir_lowering=False, debug=False, num_devices=NCORES
    )
    img_dt = mybir.dt.float8e4 if IMG_FP8 else BF16
    out_dt = U8 if OUT_U8 else BF16
    img = nc.declare_dram_parameter("img", [CD, COLS], img_dt, isOutput=False)
    st = nc.declare_dram_parameter("st", [CD, NP * 2 * F], BF16, isOutput=False)
    outq = nc.declare_dram_parameter("outq", [F, 2 * COLS], out_dt, isOutput=True)

    with TileContext(nc) as tc:
        with (
            tc.tile_pool(name="const", bufs=1) as cpool,
            tc.tile_pool(name="inp", bufs=3) as ipool,
            tc.tile_pool(name="ost", bufs=3) as opool,
            tc.tile_pool(name="ps", bufs=4, space="PSUM") as pspool,
        ):
            # stationaries: per pair two [80,80] block-diag tiles, loaded once
            st_t = cpool.tile([CD, NP * 2 * F], dtype=BF16)
            nc.sync.dma_start(out=st_t[:], in_=st[:])

            oq3 = outq[:].rearrange("p (m c) -> p m c", m=2)
            for _rep in range(reps):
                for g in range(NG):
                    csl = slice(g * GP * L, (g + 1) * GP * L)
                    it = ipool.tile([CD, GP * L], dtype=img_dt, tag="img")
                    nc.sync.dma_start(out=it[:], in_=img[:, csl])
                    if g % OG == 0:
                        otg = opool.tile([F, 2 * OG * GP * L], dtype=out_dt, tag="ost")
                        ot3 = otg[:].rearrange("p (m c) -> p m c", m=2)
                    gof = (g % OG) * GP * L
                    for j in range(GP):
                        pair = g * GP + j
                        o_ps = pspool.tile([F, 2 * L], dtype=F32, space="PSUM", tag="ps")
                        for m in range(2):
                            nc.tensor.matmul(
                                o_ps[:, m * L : (m + 1) * L],
                                lhsT=st_t[:, (pair * 2 + m) * F : (pair * 2 + m + 1) * F],
                                rhs=it[:, j * L : (j + 1) * L],
                                start=True, stop=True,
                            )
                        # quantize+evacuate: split across DVE / ACT
                        o3 = o_ps[:].rearrange("p (m c) -> p m c", m=2)
                        s_dve = slice(gof + j * L, gof + j * L + EV_DVE)
                        s_act = slice(gof + j * L + EV_DVE, gof + (j + 1) * L)
                        if OUT_U8:
                            nc.vector.tensor_scalar(
                                ot3[:, :, s_dve], o3[:, :, : EV_DVE],
                                QSCALE, QBIAS,
                                mybir.AluOpType.mult, mybir.AluOpType.add,
                            )
                            nc.scalar.activation(
                                ot3[:, :, s_act], o3[:, :, EV_DVE :],
                                mybir.ActivationFunctionType.Copy,
                                bias=QBIAS, scale=QSCALE,
                            )
                        else:
                            nc.vector.tensor_copy(
                                out=ot3[:, :, s_dve], in_=o3[:, :, : EV_DVE]
                            )
                            nc.scalar.copy(
                                out=ot3[:, :, s_act], in_=o3[:, :, EV_DVE :]
                            )
                    # out-DMA on the ACT HWDGE ring (SP ring carries input),
                    # one DMA per OG groups
                    if g % OG == OG - 1:
                        osl = slice((g - OG + 1) * GP * L, (g + 1) * GP * L)
                        nc.scalar.dma_start(out=oq3[:, :, osl], in_=ot3[:, :, :])
    nc.finalize()
    return nc


def _host_prep(exchangeability_kernel, equilibrium_kernel):
    """Tiny (H,K,20,20) eigen prep in float64 on host -> BDV [40,80],
    BDW [80,80] (block-diagonal), lam [80]."""
    ek = exchangeability_kernel.astype(np.float64)
    eq = equilibrium_kernel.astype(np.float64)
    Rm = 0.5 * (ek + np.swapaxes(ek, -1, -2))
    Rm = np.logaddexp(0.0, Rm)  # softplus
    Rm = Rm * (1.0 - np.eye(S))
    em = eq - eq.max(axis=-1, keepdims=True)
    p = np.exp(em)
    p /= p.sum(axis=-1, keepdims=True)
    Q = Rm * p[..., None, :]
    row = Q.sum(axis=-1)
    Q = Q - row[..., :, None] * np.eye(S)
    mue = (p * row).sum(axis=-1)[..., None, None]
    Q = Q / np.maximum(mue, 1e-16)
    sqrt_p = np.sqrt(p)
    inv_sqrt_p = 1.0 / sqrt_p
    Sm = sqrt_p[..., :, None] * Q * inv_sqrt_p[..., None, :]
    Sm = 0.5 * (Sm + np.swapaxes(Sm, -1, -2))
    lam, U = np.linalg.eigh(Sm)  # (H,K,S), (H,K,S,S)

    BDV = np.zeros((H * S, F), dtype=np.float64)
    BDW = np.zeros((F, F), dtype=np.float64)
    for h in range(H):
        for k in range(K):
            c = h * K * S + k * S
            BDV[h * S : (h + 1) * S, c : c + S] = inv_sqrt_p[h, k][:, None] * U[h, k]
            BDW[c : c + S, c : c + S] = (sqrt_p[h, k][:, None] * U[h, k]).T
    return BDV, BDW, lam.reshape(F)


def kernel(inputs, rate_indices, tau_kernel, exchangeability_kernel, equilibrium_kernel):
    inputs = np.asarray(inputs, dtype=np.float32)
    rate_indices = np.asarray(rate_indices)
    tau_kernel = np.asarray(tau_kernel, dtype=np.float64)

    BDV, BDW, lam = _host_prep(
        np.asarray(exchangeability_kernel), np.asarray(equilibrium_kernel)
    )
    # tau/E/P_comb on host in f64
    h_idx = np.arange(H)[None, :]
    tau = np.logaddexp(0.0, tau_kernel[h_idx, rate_indices])     # (B,H)
    lam_hb = lam.reshape(H, K * S)
    E = np.exp(tau[:, :, None] * lam_hb[None]).reshape(B, F)     # (B,80)
    # pc[b] = BDV @ diag(E_b) @ BDW   -> (B, 40, 80)
    pc = np.matmul(BDV[None] * E[:, None, :], BDW)
    pc_bf = pc.astype(NPBF16)

    if "nc" not in _NC_CACHE:
        _NC_CACHE["nc"] = build_nc()
    nc = _NC_CACHE["nc"]

    in_maps = []
    for c in range(NCORES):
        bsl = slice(c * BPC, (c + 1) * BPC)
        # one-hot moving image [80, 32768]: partition = parity*40+h*20+z
        arr = inputs[bsl].reshape(NP, 2, L, H, S)
        imgc = np.ascontiguousarray(
            arr.transpose(1, 3, 4, 0, 2).reshape(CD, COLS)
        )
        if IMG_FP8:
            img_np = (
                (imgc != 0).astype(np.uint8) * np.uint8(0x38)
            ).view(NPF8E4)
        else:
            img_np = (
                (imgc != 0).astype(np.uint16) * np.uint16(0x3F80)
            ).view(NPBF16)
        # stationaries [80, 64*2*80] bf16: per pair two [80,80] block-diags
        pcs = pc_bf[bsl]                      # (128, 40, 80)
        stc = np.zeros((CD, NP, 2, F), dtype=NPBF16)
        even = pcs[0::2].transpose(1, 0, 2)   # (40, 64, 80)
        odd = pcs[1::2].transpose(1, 0, 2)
        stc[0:40, :, 0, 0:40] = even[:, :, 0:40]
        stc[0:40, :, 1, 0:40] = even[:, :, 40:80]
        stc[40:80, :, 0, 40:80] = odd[:, :, 0:40]
        stc[40:80, :, 1, 40:80] = odd[:, :, 40:80]
        in_maps.append(
            {
                "img": img_np,
                "st": np.ascontiguousarray(stc.reshape(CD, NP * 2 * F)),
            }
        )

    _NC_CACHE["in_maps"] = in_maps
    res = run_bass_kernel_spmd(nc, in_maps, core_ids=list(range(NCORES)))

    out = np.empty((B, L, H, K, S), dtype=np.float32)
    for c in range(NCORES):
        o = res.results[c]["outq"]            # (80, 2*32768)
        o5 = o.reshape(2, 40, 2, NP, L)       # (parity, q, m=h, pair, l)
        if OUT_U8:
            xf = (o5.astype(np.float32) - DEC_BIAS) * (1.0 / QSCALE)
        else:
            xf = o5.astype(np.float32)
        # -> (pair, parity, l, m, q) -> (B/8, L, H, K, S)
        out[c * BPC : (c + 1) * BPC] = (
            xf.transpose(3, 0, 4, 2, 1)
            .reshape(BPC, L, H, K, S)
        )
    return out


# revision 5
# speedup vs baseline: 35.5836x; 35.5836x over previous
"""Distributed Trainium2 kernel for nn_AncProbsLayer.

Math (reference):
    tau[b,h]  = softplus(tau_kernel[h, rate_indices[b,h]])
    R,p,Q     from tiny (H,K,20,20) kernels; Sm = D^1/2 Q D^-1/2; lam,U = eigh(Sm)
    P[b,h,k]  = D^-1/2 U diag(exp(tau*lam)) U^T D^1/2
    out       = einsum('blhz,bhkzs->blhks', inputs, P)

Key structural facts exploited:
  * `inputs` is ONE-HOT, so the contraction has exactly one nonzero term
    per h-block: out rows are just rows of P_comb[b] = BDV diag(E_b) BDW.
    A single bf16 matmul reproduces them to ~0.2% (gate is 2e-2
    absmax-relative); no hi/lo splitting is needed, and the one-hot moving
    operand is EXACT in fp8 (values 0/1), halving input DMA bytes.
  * All the tiny eigen/tau/P_comb math runs on HOST (float64) — only the
    streaming gather-matmul runs on device. Per pair of batches (even b at
    image partitions 0-39, odd at 40-79) two matmuls with an [80,80]
    block-diagonal stationary (pc_even ⊕ pc_odd, split in two 40-feature
    halves) produce the full [80-feature, 512-position] output tile.
  * Output is uint8-quantized on-chip (u8 = 250*x + 4.5, decoded on host);
    P entries lie in [~-0.01, 0.99] so the quant error is ~0.2% absmax.

Distribution: data-parallel over batch B across 8 cores (128 b each), no
collectives. Per core per pass: in-DMA 2.62 MB (fp8 one-hot image), out-DMA
5.24 MB (u8), PE 2x64 matmuls x 512 moving rows = 65536 cycles ~= 27.3 us
@2.4 GHz -- the expected bottleneck. PSUM is evacuated in per-pair
[80,1024] tiles split across DVE/ACT/Pool.
"""

import numpy as np
import ml_dtypes

import concourse.bass as bass
import concourse.bacc as bacc
import concourse.mybir as mybir
from concourse.tile import TileContext
from concourse.bass_utils import run_bass_kernel_spmd

# Problem constants (hardcoded per the harness contract)
B, L, H, K, S = 1024, 512, 2, 2, 20
NCORES = 8
BPC = B // NCORES          # 128 batches per core
NP = BPC // 2              # 64 even/odd pairs per core
COLS = NP * L              # 32768 image columns per core
F = H * K * S              # 80 output features
CD = 2 * H * S             # 80 contraction rows (even 0-39, odd 40-79)
GP = 8                     # pairs per DMA group
NG = NP // GP              # 8 groups

F32 = mybir.dt.float32
BF16 = mybir.dt.bfloat16
U8 = mybir.dt.uint8
NPBF16 = np.dtype(ml_dtypes.bfloat16)
NPF8E4 = np.dtype(ml_dtypes.float8_e4m3)

IMG_FP8 = False             # one-hot moving image dtype: fp8e4 (exact) vs bf16
OUT_U8 = True              # on-chip u8 quantization of the output
QSCALE = 250.0
QBIAS = 4.5
DEC_BIAS = 4.25            # host decode offset (covers trunc or RNE convert)

# evac column split within each 512-col half: DVE | ACT (GPSIMD/Pool
# cannot read PSUM)
EV_DVE = 328
EV_ACT = 184               # 328 + 184 = 512
OG = 2                     # groups per out-DMA

_NC_CACHE = {}


def build_nc(reps=1):
    # reps>1 repeats the main stream inside one NEFF (benchmarking only:
    # (wall[R] - wall[1])/(R-1) cancels dispatch overhead exactly)
    nc = bacc.Bacc(
        "TRN2", target_bir_lowering=False, debug=False, num_devices=NCORES
    )
    img_dt = mybir.dt.float8e4 if IMG_FP8 else BF16
    out_dt = U8 if OUT_U8 else BF16
    img = nc.declare_dram_parameter("img", [CD, COLS], img_dt, isOutput=False)
    st = nc.declare_dram_parameter("st", [CD, NP * 2 * F], BF16, isOutput=False)
    outq = nc.declare_dram_parameter("outq", [F, 2 * COLS], out_dt, isOutput=True)

    with TileContext(nc) as tc:
        with (
            tc.tile_pool(name="const", bufs=1) as cpool,
            tc.tile_pool(name="inp", bufs=3) as ipool,
            tc.tile_pool(name="ost", bufs=3) as opool,
            tc.tile_pool(name="ps", bufs=4, space="PSUM") as pspool,
        ):
            # stationaries: per pair two [80,80] block-diag tiles, loaded once
            st_t = cpool.tile([CD, NP * 2 * F], dtype=BF16)
            nc.sync.dma_start(out=st_t[:], in_=st[:])

            oq3 = outq[:].rearrange("p (m c) -> p m c", m=2)
            for _rep in range(reps):
                for g in range(NG):
                    csl = slice(g * GP * L, (g + 1) * GP * L)
                    it = ipool.tile([CD, GP * L], dtype=img_dt, tag="img")
                    nc.sync.dma_start(out=it[:], in_=img[:, csl])
                    if g % OG == 0:
                        otg = opool.tile([F, 2 * OG * GP * L], dtype=out_dt, tag="ost")
                        ot3 = otg[:].rearrange("p (m c) -> p m c", m=2)
                    gof = (g % OG) * GP * L
                    for j in range(GP):
                        pair = g * GP + j
                        o_ps = pspool.tile([F, 2 * L], dtype=F32, space="PSUM", tag="ps")
                        for m in range(2):
                            nc.tensor.matmul(
                                o_ps[:, m * L : (m + 1) * L],
                                lhsT=st_t[:, (pair * 2 + m) * F : (pair * 2 + m + 1) * F],
                                rhs=it[:, j * L : (j + 1) * L],
                                start=True, stop=True,
                            )
                        # quantize+evacuate: split across DVE / ACT
                        o3 = o_ps[:].rearrange("p (m c) -> p m c", m=2)
                        s_dve = slice(gof + j * L, gof + j * L + EV_DVE)
                        s_act = slice(gof + j * L + EV_DVE, gof + (j + 1) * L)
                        if OUT_U8:
                            nc.vector.tensor_scalar(
                                ot3[:, :, s_dve], o3[:, :, : EV_DVE],
                                QSCALE, QBIAS,
                                mybir.AluOpType.mult, mybir.AluOpType.add,
                            )
                            nc.scalar.activation(
                                ot3[:, :, s_act], o3[:, :, EV_DVE :],
                                mybir.ActivationFunctionType.Copy,
                                bias=QBIAS, scale=QSCALE,
                            )
                        else:
                            nc.vector.tensor_copy(
                                out=ot3[:, :, s_dve], in_=o3[:, :, : EV_DVE]
                            )
                            nc.scalar.copy(
                                out=ot3[:, :, s_act], in_=o3[:, :, EV_DVE :]
                            )
                    # out-DMA on the ACT HWDGE ring (SP ring carries input),
                    # one DMA per OG groups
                    if g % OG == OG - 1:
                        osl = slice((g - OG + 1) * GP * L, (g + 1) * GP * L)
                        nc.scalar.dma_start(out=oq3[:, :, osl], in_=ot3[:, :, :])
    nc.finalize()
    return nc


def _host_prep(exchangeability_kernel, equilibrium_kernel):
    """Tiny (H,K,20,20) eigen prep in float64 on host -> BDV [40,80],
    BDW [80,80] (block-diagonal), lam [80]."""
    ek = exchangeability_kernel.astype(np.float64)
    eq = equilibrium_kernel.astype(np.float64)
    Rm = 0.5 * (ek + np.swapaxes(ek, -1, -2))
    Rm = np.logaddexp(0.0, Rm)  # softplus
    Rm = Rm * (1.0 - np.eye(S))
    em = eq - eq.max(axis=-1, keepdims=True)
    p = np.exp(em)
    p /= p.sum(axis=-1, keepdims=True)
    Q = Rm * p[..., None, :]
    row = Q.sum(axis=-1)
    Q = Q - row[..., :, None] * np.eye(S)
    mue = (p * row).sum(axis=-1)[..., None, None]
    Q = Q / np.maximum(mue, 1e-16)
    sqrt_p = np.sqrt(p)
    inv_sqrt_p = 1.0 / sqrt_p
    Sm = sqrt_p[..., :, None] * Q * inv_sqrt_p[..., None, :]
    Sm = 0.5 * (Sm + np.swapaxes(Sm, -1, -2))
    lam, U = np.linalg.eigh(Sm)  # (H,K,S), (H,K,S,S)

    BDV = np.zeros((H * S, F), dtype=np.float64)
    BDW = np.zeros((F, F), dtype=np.float64)
    for h in range(H):
        for k in range(K):
            c = h * K * S + k * S
            BDV[h * S : (h + 1) * S, c : c + S] = inv_sqrt_p[h, k][:, None] * U[h, k]
            BDW[c : c + S, c : c + S] = (sqrt_p[h, k][:, None] * U[h, k]).T
    return BDV, BDW, lam.reshape(F)


def kernel(inputs, rate_indices, tau_kernel, exchangeability_kernel, equilibrium_kernel):
    inputs = np.asarray(inputs, dtype=np.float32)
    rate_indices = np.asarray(rate_indices)
    tau_kernel = np.asarray(tau_kernel, dtype=np.float64)

    BDV, BDW, lam = _host_prep(
        np.asarray(exchangeability_kernel), np.asarray(equilibrium_kernel)
    )
    # tau/E/P_comb on host in f64
    h_idx = np.arange(H)[None, :]
    tau = np.logaddexp(0.0, tau_kernel[h_idx, rate_indices])     # (B,H)
    lam_hb = lam.reshape(H, K * S)
    E = np.exp(tau[:, :, None] * lam_hb[None]).reshape(B, F)     # (B,80)
    # pc[b] = BDV @ diag(E_b) @ BDW   -> (B, 40, 80)
    pc = np.matmul(BDV[None] * E[:, None, :], BDW)
    pc_bf = pc.astype(NPBF16)

    if "nc" not in _NC_CACHE:
        _NC_CACHE["nc"] = build_nc()
    nc = _NC_CACHE["nc"]

    in_maps = []
    for c in range(NCORES):
        bsl = slice(c * BPC, (c + 1) * BPC)
        # one-hot moving image [80, 32768]: partition = parity*40+h*20+z
        arr = inputs[bsl].reshape(NP, 2, L, H, S)
        imgc = np.ascontiguousarray(
            arr.transpose(1, 3, 4, 0, 2).reshape(CD, COLS)
        )
        if IMG_FP8:
            img_np = (
                (imgc != 0).astype(np.uint8) * np.uint8(0x38)
            ).view(NPF8E4)
        else:
            img_np = (
                (imgc != 0).astype(np.uint16) * np.uint16(0x3F80)
            ).view(NPBF16)
        # stationaries [80, 64*2*80] bf16: per pair two [80,80] block-diags
        pcs = pc_bf[bsl]                      # (128, 40, 80)
        stc = np.zeros((CD, NP, 2, F), dtype=NPBF16)
        even = pcs[0::2].transpose(1, 0, 2)   # (40, 64, 80)
        odd = pcs[1::2].transpose(1, 0, 2)
        stc[0:40, :, 0, 0:40] = even[:, :, 0:40]
        stc[0:40, :, 1, 0:40] = even[:, :, 40:80]
        stc[40:80, :, 0, 40:80] = odd[:, :, 0:40]
        stc[40:80, :, 1, 40:80] = odd[:, :, 40:80]
        in_maps.append(
            {
                "img": img_np,
                "st": np.ascontiguousarray(stc.reshape(CD, NP * 2 * F)),
            }
        )

    _NC_CACHE["in_maps"] = in_maps
    res = run_bass_kernel_spmd(nc, in_maps, core_ids=list(range(NCORES)))

    out = np.empty((B, L, H, K, S), dtype=np.float32)
    for c in range(NCORES):
        o = res.results[c]["outq"]            # (80, 2*32768)
        o5 = o.reshape(2, 40, 2, NP, L)       # (parity, q, m=h, pair, l)
        if OUT_U8:
            xf = (o5.astype(np.float32) - DEC_BIAS) * (1.0 / QSCALE)
        else:
            xf = o5.astype(np.float32)
        # -> (pair, parity, l, m, q) -> (B/8, L, H, K, S)
        out[c * BPC : (c + 1) * BPC] = (
            xf.transpose(3, 0, 4, 2, 1)
            .reshape(BPC, L, H, K, S)
        )
    return out
